# revision 1
# baseline (speedup 1.0000x reference)
"""Trainium2 Bass kernel for nn_CrossModalFusionModel (sparse sliding-window
cross-attention, 2 modules: image<-text and text<-image).

Sharding: head-parallel tensor parallelism over 8 NeuronCores. Core h owns
attention head h (dh=128) of BOTH modules: it computes its head's Q/K/V with
host-folded projection weights (input-proj and attention-proj chains collapse
into one matmul), runs full-sequence banded attention for that head, and emits
a full-D o-projection partial plus its D-slice of the residual projection.
The host sums the 8 partials (the unshard step). No collectives.

Everything on-device lives in transposed [D, seq] layout so scores/AV/o-proj
chain through the PE without any on-chip transposes; the host pre-transposes
inputs and post-transposes outputs.

The reference's zero-pad attention slots (up to window tokens of kb/vb at the
sequence edges) collapse into ONE virtual pad column per query with
multiplicative weight n_pad(i), since all pad slots share the score q.kb.
"""

import math

import numpy as np
import ml_dtypes

N = 512          # tokens / patches
DM = 1024        # d_model
DH = 128         # head dim
NT = N // 128    # 4 j-tiles
C_IMG = 1024
C_TXT = 768
WINDOW = 64
NCORES = 8

# compute dtype knob: "f32r" | "f16" | "bf16" | "f32"
COMPUTE_DTYPE = "f32r"

_prog_cache = {}
LAST_RESULT = {}


def _mybir_cd(cd):
    from concourse import mybir
    return {
        "f32r": mybir.dt.float32r,
        "f16": mybir.dt.float16,
        "bf16": mybir.dt.bfloat16,
        "f32": mybir.dt.float32,
    }[cd]


def _np_cd(cd):
    return {
        "f32r": np.float32,
        "f16": np.float16,
        "bf16": ml_dtypes.bfloat16,
        "f32": np.float32,
    }[cd]


def _host_cd(x, cd):
    """Convert a float64 host array to the wire format of compute dtype cd."""
    x = np.ascontiguousarray(x)
    if cd == "f32r":
        x = x.astype(np.float32)
        hi = x.astype(ml_dtypes.bfloat16).astype(np.float32)
        lo = (x - hi).astype(ml_dtypes.bfloat16).astype(np.float32)
        return hi + lo
    return x.astype(_np_cd(cd))


def _build_program(cd):
    import concourse.bass as bass
    import concourse.tile as tile
    from concourse import bacc, mybir

    f32 = mybir.dt.float32
    CD = _mybir_cd(cd)
    Exp = mybir.ActivationFunctionType.Exp

    nc = bacc.Bacc("TRN2", target_bir_lowering=False, debug=False,
                   num_devices=NCORES)

    def din(name, shape, dt=CD):
        return nc.dram_tensor(name, shape, dt, kind="ExternalInput")

    def dout(name, shape, dt=f32):
        return nc.dram_tensor(name, shape, dt, kind="ExternalOutput")

    # Activations (transposed) and masks are identical on every core.
    xT_img = din("xT_img", [C_IMG, N])
    xT_txt = din("xT_txt", [C_TXT, N])
    maskM = din("maskM", [128, NT * N])
    maskP = din("maskP", [1, N])

    # Per-core (per-head) folded weights.
    mods = {}
    for m, cq, cc in (("ia", C_IMG, C_TXT), ("ta", C_TXT, C_IMG)):
        mods[m] = dict(
            wqT=din(f"wqT_{m}", [cq, DH]),
            wkT=din(f"wkT_{m}", [cc, DH]),
            wvT=din(f"wvT_{m}", [cc, DH]),
            woT=din(f"woT_{m}", [DH, DM]),
            kbcol=din(f"kbcol_{m}", [DH, 1]),
            vbpad=din(f"vbpad_{m}", [1, DH]),
            bvrow=din(f"bvrow_{m}", [1, DH]),
            bq=din(f"bq_{m}", [DH, 1], f32),
            bk=din(f"bk_{m}", [DH, 1], f32),
            po=dout(f"po_{m}", [DM, N]),
            xr=dout(f"xr_{m}", [DH, N]),
        )
    rwT_img = din("rwT_img", [C_IMG, DH])   # ip_w D-slice (residual)
    rwT_txt = din("rwT_txt", [C_TXT, DH])   # tp_w D-slice
    brx = din("brx", [DH, 1], f32)          # ip_b slice
    brt = din("brt", [DH, 1], f32)          # tp_b slice
    ones_c = din("ones_c", [128, 1])
    ones_r = din("ones_r", [1, 128])

    with tile.TileContext(nc) as tc:
        with tc.tile_pool(name="consts", bufs=1) as consts, \
             tc.tile_pool(name="work", bufs=3) as work, \
             tc.tile_pool(name="epool", bufs=3) as epool, \
             tc.tile_pool(name="small", bufs=2) as small, \
             tc.tile_pool(name="ps_st", bufs=2, space="PSUM") as ps_st, \
             tc.tile_pool(name="ps_small", bufs=1, space="PSUM") as ps_small, \
             tc.tile_pool(name="ps_acc", bufs=4, space="PSUM") as ps_acc:

            def load3(name, dram, c, n):
                t = consts.tile([128, c // 128, n], CD, tag=name)
                nc.sync.dma_start(
                    t[:], dram.ap().rearrange("(c p) n -> p c n", p=128))
                return t

            xi = load3("xi", xT_img, C_IMG, N)
            xt = load3("xt", xT_txt, C_TXT, N)
            rwi = load3("rwi", rwT_img, C_IMG, DH)
            rwt = load3("rwt", rwT_txt, C_TXT, DH)

            mM = consts.tile([128, NT * N], CD, tag="mM")
            nc.sync.dma_start(mM[:], maskM[:])
            mP = consts.tile([1, N], CD, tag="mP")
            nc.sync.dma_start(mP[:], maskP[:])

            ones_col = consts.tile([128, 1], CD, tag="ones_col")
            nc.sync.dma_start(ones_col[:], ones_c[:])
            ones_row = consts.tile([1, 128], CD, tag="ones_row")
            nc.sync.dma_start(ones_row[:], ones_r[:])

            sb = {}
            for m, cq, cc in (("ia", C_IMG, C_TXT), ("ta", C_TXT, C_IMG)):
                d = mods[m]
                sb[m] = dict(
                    wq=load3(f"wq_{m}", d["wqT"], cq, DH),
                    wk=load3(f"wk_{m}", d["wkT"], cc, DH),
                    wv=load3(f"wv_{m}", d["wvT"], cc, DH),
                )
                wo = consts.tile([DH, DM], CD, tag=f"wo_{m}")
                nc.sync.dma_start(wo[:], d["woT"][:])
                kbc = consts.tile([DH, 1], CD, tag=f"kbc_{m}")
                nc.sync.dma_start(kbc[:], d["kbcol"][:])
                vbp = consts.tile([1, DH], CD, tag=f"vbp_{m}")
                nc.sync.dma_start(vbp[:], d["vbpad"][:])
                bvr = consts.tile([1, DH], CD, tag=f"bvr_{m}")
                nc.sync.dma_start(bvr[:], d["bvrow"][:])
                bq = consts.tile([DH, 1], f32, tag=f"bq_{m}")
                nc.sync.dma_start(bq[:], d["bq"][:])
                bk = consts.tile([DH, 1], f32, tag=f"bk_{m}")
                nc.sync.dma_start(bk[:], d["bk"][:])
                sb[m].update(wo=wo, kbc=kbc, vbp=vbp, bvr=bvr, bq=bq, bk=bk)
            bxi = consts.tile([DH, 1], f32, tag="bxi")
            nc.sync.dma_start(bxi[:], brx[:])
            bxt = consts.tile([DH, 1], f32, tag="bxt")
            nc.sync.dma_start(bxt[:], brt[:])

            def projT(w3, x3, nct, bias_col, tag, out_dt=CD):
                """out^T [128, N] = (x @ W^T)^T + bias, via contraction tiles."""
                ps = ps_acc.tile([128, N], f32, tag="acc")
                for ct in range(nct):
                    nc.tensor.matmul(ps[:], w3[:, ct, :], x3[:, ct, :],
                                     start=(ct == 0), stop=(ct == nct - 1))
                out = work.tile([128, N], out_dt, tag="sb_" + tag)
                nc.vector.tensor_scalar_add(out[:], ps[:], bias_col[:])
                return out

            def vproj_nat(x3, w3, nct, bvr, tag):
                """V natural [j, d] in one [128, NT*128] tile (jt at free jt*128)."""
                ps = ps_acc.tile([128, NT * DH], f32, tag="acc")
                for jt in range(NT):
                    blk = ps[:, jt * DH:(jt + 1) * DH]
                    for ct in range(nct):
                        nc.tensor.matmul(
                            blk, x3[:, ct, jt * 128:(jt + 1) * 128],
                            w3[:, ct, :], start=(ct == 0), stop=False)
                    nc.tensor.matmul(blk, ones_row[:, :], bvr[:],
                                     start=False, stop=True)
                out = work.tile([128, NT * DH], CD, tag="sb_" + tag)
                nc.vector.tensor_copy(out[:], ps[:])
                return out

            def residT(w3, x3, nct, bias_col, dram, tag):
                ps = ps_acc.tile([128, N], f32, tag="acc")
                for ct in range(nct):
                    nc.tensor.matmul(ps[:], w3[:, ct, :], x3[:, ct, :],
                                     start=(ct == 0), stop=(ct == nct - 1))
                out = work.tile([128, N], f32, tag="sb_" + tag)
                nc.vector.tensor_scalar_add(out[:], ps[:], bias_col[:])
                nc.sync.dma_start(dram[:], out[:])

            for m, xq3, nq, xc3, ncc in (("ia", xi, 8, xt, 6),
                                         ("ta", xt, 6, xi, 8)):
                s = sb[m]
                d = mods[m]
                qT = projT(s["wq"], xq3, nq, s["bq"], "q")
                kT = projT(s["wk"], xc3, ncc, s["bk"], "k")
                vN = vproj_nat(xc3, s["wv"], ncc, s["bvr"], "v")

                # scores S^T per j-tile, exp, band-mask
                eTm = epool.tile([128, NT * N], CD, tag="eTm")
                for jt in range(NT):
                    st = ps_st.tile([128, N], f32, tag="st")
                    nc.tensor.matmul(st[:], kT[:, jt * 128:(jt + 1) * 128],
                                     qT[:], start=True, stop=True)
                    eT = epool.tile([128, N], CD, tag="eT")
                    nc.scalar.activation(eT[:], st[:], Exp)
                    nc.vector.tensor_mul(eTm[:, jt * N:(jt + 1) * N], eT[:],
                                         mM[:, jt * N:(jt + 1) * N])
                # virtual pad column (score q.kb, weight n_pad)
                sp = ps_small.tile([1, N], f32, tag="smallp")
                nc.tensor.matmul(sp[:], s["kbc"][:], qT[:], start=True,
                                 stop=True)
                eP = small.tile([1, N], CD, tag="eP")
                nc.scalar.activation(eP[:], sp[:], Exp)
                ePm = small.tile([1, N], CD, tag="ePm")
                nc.vector.tensor_mul(ePm[:], eP[:], mP[:])

                # softmax denominators
                ssum = ps_small.tile([1, N], f32, tag="smallp2")
                for jt in range(NT):
                    nc.tensor.matmul(ssum[:], ones_col[:],
                                     eTm[:, jt * N:(jt + 1) * N],
                                     start=(jt == 0), stop=False)
                nc.tensor.matmul(ssum[:], ones_col[0:1, :], ePm[:],
                                 start=False, stop=True)
                rinv = small.tile([1, N], CD, tag="rinv")
                with nc.allow_low_precision(
                        reason="softmax 1/denom feeds a CD-dtype matmul; "
                               "CD is >= fp16 and denom is O(1-100)"):
                    nc.vector.reciprocal(rinv[:], ssum[:])

                # O^T = V^T E^T (+ pad)
                oT = ps_acc.tile([128, N], f32, tag="acc")
                for jt in range(NT):
                    nc.tensor.matmul(oT[:], vN[:, jt * DH:(jt + 1) * DH],
                                     eTm[:, jt * N:(jt + 1) * N],
                                     start=(jt == 0), stop=False)
                nc.tensor.matmul(oT[:], s["vbp"][:], ePm[:], start=False,
                                 stop=True)

                # normalize: broadcast rinv to 128 partitions via PE
                rbc = ps_acc.tile([128, N], f32, tag="acc")
                nc.tensor.matmul(rbc[:], ones_row[:], rinv[:], start=True,
                                 stop=True)
                rbc_sb = work.tile([128, N], f32, tag="rbc_sb")
                nc.vector.tensor_copy(rbc_sb[:], rbc[:])
                onorm = work.tile([128, N], CD, tag="onorm")
                nc.vector.tensor_mul(onorm[:], oT[:], rbc_sb[:])

                # o-projection partial: po[dt*128:, :] = wo_h[:, dt].T @ onorm
                for dt_i in range(DM // 128):
                    po = ps_acc.tile([128, N], f32, tag="acc")
                    nc.tensor.matmul(po[:],
                                     s["wo"][:, dt_i * 128:(dt_i + 1) * 128],
                                     onorm[:], start=True, stop=True)
                    po_sb = work.tile([128, N], f32, tag="po_sb")
                    nc.vector.tensor_copy(po_sb[:], po[:])
                    nc.sync.dma_start(d["po"][dt_i * 128:(dt_i + 1) * 128, :],
                                      po_sb[:])

            # residual D-slices (fp32 out)
            residT(rwi, xi, 8, bxi, mods["ia"]["xr"], "xri")
            residT(rwt, xt, 6, bxt, mods["ta"]["xr"], "xrt")

    nc.compile()
    return nc


def _masks(cd):
    i = np.arange(N)
    j = np.arange(N)
    band = (j[:, None] >= i[None, :] - WINDOW // 2) & \
           (j[:, None] <= i[None, :] + WINDOW // 2 + 1)   # [j, i]
    length = band.sum(axis=0)
    npad = np.maximum(0, WINDOW - length)
    mM = band.astype(np.float64).reshape(NT, 128, N).transpose(1, 0, 2) \
             .reshape(128, NT * N)
    mP = npad.astype(np.float64)[None, :]
    return _host_cd(mM, cd), _host_cd(mP, cd)


def kernel(**inputs):
    from concourse.bass_utils import run_bass_kernel_spmd

    cd = COMPUTE_DTYPE
    if cd not in _prog_cache:
        _prog_cache[cd] = _build_program(cd)
    nc = _prog_cache[cd]

    f8 = lambda x: np.asarray(x, dtype=np.float64)
    images = f8(inputs["images"])[0]        # [N, 1024]
    caps = f8(inputs["capitions"])[0]       # [N, 768]
    ip_w, ip_b = f8(inputs["ip_w"]), f8(inputs["ip_b"])
    tp_w, tp_b = f8(inputs["tp_w"]), f8(inputs["tp_b"])

    sc = 1.0 / math.sqrt(DH)
    mM, mP = _masks(cd)
    xTi = _host_cd(images.T, cd)
    xTt = _host_cd(caps.T, cd)

    in_maps = []
    for h in range(NCORES):
        sl = slice(h * DH, (h + 1) * DH)
        im = {
            "xT_img": xTi, "xT_txt": xTt, "maskM": mM, "maskP": mP,
            "rwT_img": _host_cd(ip_w[sl].T, cd),
            "rwT_txt": _host_cd(tp_w[sl].T, cd),
            "ones_c": _host_cd(np.ones((128, 1)), cd),
            "ones_r": _host_cd(np.ones((1, 128)), cd),
            "brx": np.ascontiguousarray(ip_b[sl, None], dtype=np.float32),
            "brt": np.ascontiguousarray(tp_b[sl, None], dtype=np.float32),
        }
        for m, pw, pb, cw, cb in (("ia", ip_w, ip_b, tp_w, tp_b),
                                  ("ta", tp_w, tp_b, ip_w, ip_b)):
            qw, qb = f8(inputs[f"{m}_qw"]), f8(inputs[f"{m}_qb"])
            kw, kb = f8(inputs[f"{m}_kw"]), f8(inputs[f"{m}_kb"])
            vw, vb = f8(inputs[f"{m}_vw"]), f8(inputs[f"{m}_vb"])
            ow = f8(inputs[f"{m}_ow"])
            im[f"wqT_{m}"] = _host_cd(((qw[sl] @ pw) * sc).T, cd)
            im[f"bq_{m}"] = ((qw[sl] @ pb + qb[sl]) * sc)[:, None] \
                .astype(np.float32)
            im[f"wkT_{m}"] = _host_cd((kw[sl] @ cw).T, cd)
            im[f"bk_{m}"] = (kw[sl] @ cb + kb[sl])[:, None].astype(np.float32)
            im[f"wvT_{m}"] = _host_cd((vw[sl] @ cw).T, cd)
            im[f"bvrow_{m}"] = _host_cd((vw[sl] @ cb + vb[sl])[None, :], cd)
            im[f"woT_{m}"] = _host_cd(ow[:, sl].T, cd)
            im[f"kbcol_{m}"] = _host_cd(kb[sl, None], cd)
            im[f"vbpad_{m}"] = _host_cd(vb[sl][None, :], cd)
        in_maps.append(im)

    res = run_bass_kernel_spmd(nc, in_maps, list(range(NCORES)))
    LAST_RESULT["res"] = res

    outs = []
    for m in ("ia", "ta"):
        acc = np.zeros((DM, N), dtype=np.float64)
        for h in range(NCORES):
            r = res.results[h]
            acc += r[f"po_{m}"].astype(np.float64)
            acc[h * DH:(h + 1) * DH] += r[f"xr_{m}"].astype(np.float64)
        acc += f8(inputs["ia_ob" if m == "ia" else "ta_ob"])[:, None]
        outs.append(np.ascontiguousarray(acc.T[None]).astype(np.float32))
    return outs[0], outs[1]



# revision 11
# speedup vs baseline: 1.8148x; 1.8148x over previous
"""Trainium2 Bass kernel for nn_CrossModalFusionModel (sliding-window
cross-attention, 2 modules: image<-text and text<-image).

Sharding v2: module-split tensor parallelism over 8 NeuronCores. Cores 0-3
own the image-attention module (2 heads each), cores 4-7 the text-attention
module. Each core computes its 2 heads' folded Q/K/V projections (input-proj
chain collapsed into one matmul), banded sliding-window attention (scores /
softmax / AV restricted to 128x128 blocks on the 66-wide diagonal band), and
a full-D o-projection partial with the residual input-projection folded into
the same PSUM accumulation. The host sums the 4 partials per module.

Precision: f16 weights/activations (PE full rate, half the DMA bytes of f32);
bf16 for the exp/E path (exp(score) can exceed f16 max) and for 1/denom
(avoids f16 underflow); f32 PSUM accumulation everywhere.

Bias algebra: the folded k-bias adds a per-query constant to every real score
and the pad slots can be shifted to match (softmax-invariant), so K carries
no bias; the pad-slot score becomes q.(kb - kbf), one [1,N] matmul. The
folded v-bias contributes vbf + (vb - vbf) * (npad * exp(sp) / denom) to the
normalized per-head output; the constant vbf part goes to the host, the
rank-1 part is one accumulating matmul into the AV PSUM per head.
"""

import math

import numpy as np
import ml_dtypes

N = 512          # tokens / patches (both modalities)
DM = 1024        # d_model
DH = 128         # head dim
NT = N // 128    # 4 tiles of 128
C_IMG = 1024
C_TXT = 768
WINDOW = 64
NCORES = 8
NCT = 8          # contraction tiles (txt side zero-padded 768 -> 1024)

F16 = np.float16
BF16 = ml_dtypes.bfloat16

# j-subtile coverage per query i-tile: queries [128it, 128it+128) attend
# j in [128it-32, 128it+160), i.e. aligned j-tiles {it-1, it, it+1} clamped.
# 3 slots per it; duplicate a real jsub in dead slots (mask zeroes them).
JSUB = [(0, 1, 1), (0, 1, 2), (1, 2, 3), (2, 3, 3)]
NSLOT = 3
SHIFT = -10.0     # exp(score + SHIFT): softmax-invariant f16 range guard

_prog_cache = {}
LAST_RESULT = {}
COMPUTE_DTYPE = "f16+bf16"   # fixed; kept for the test harness banner


def _build_program():
    import concourse.tile as tile
    from concourse import bacc, mybir

    f32 = mybir.dt.float32
    f16 = mybir.dt.float16
    bf16 = mybir.dt.bfloat16
    Exp = mybir.ActivationFunctionType.Exp
    Ident = mybir.ActivationFunctionType.Identity
    Copy = mybir.ActivationFunctionType.Copy

    nc = bacc.Bacc("TRN2", target_bir_lowering=False, debug=False,
                   num_devices=NCORES)

    def din(name, shape, dt=f16):
        return nc.dram_tensor(name, shape, dt, kind="ExternalInput")

    # q-side activations, chunk-permuted so this core's 2 residual
    # contraction chunks sit at chunk indices 0 and 1.
    actQ = din("actQ", [128, NCT, N])
    actC = din("actC", [128, NCT, N])
    wq = din("wq", [128, NCT, 2 * DH])      # folded + scaled, rows match actQ
    wk = din("wk", [128, NCT, 2 * DH])
    wv = din("wv", [128, NCT, 2 * DH])
    worw = din("worw", [128, 4, DM])        # wo head0, wo head1, rw ch0, rw ch1
    masks = din("masks", [128, NT * NSLOT, 128], f16)
    mP = din("mP", [1, N], f32)             # npad per query
    qb = din("qb", [128, 3], f32)   # col 2: exp shift const           # per-head folded q bias
    smalls = din("smalls", [128, 3], f16)  # onesc | kbd h0 | kbd h1
    rows = din("rows", [1, 3 * 128], f16)  # ones row | vbd h0 | vbd h1
    po = nc.dram_tensor("po", [128, DM // 128, N], f16, kind="ExternalOutput")

    with tile.TileContext(nc) as tc:
        with tc.tile_pool(name="consts", bufs=1) as consts, \
             tc.tile_pool(name="work", bufs=3) as work, \
             tc.tile_pool(name="po_sb", bufs=3) as po_pool, \
             tc.tile_pool(name="small", bufs=2) as small, \
             tc.tile_pool(name="ps_big", bufs=2, space="PSUM") as ps_big, \
             tc.tile_pool(name="ps_st", bufs=2, space="PSUM") as ps_st, \
             tc.tile_pool(name="ps_att", bufs=2, space="PSUM") as ps_att, \
             tc.tile_pool(name="ps_ssum", bufs=1, space="PSUM") as ps_ssum, \
             tc.tile_pool(name="ps_sp", bufs=1, space="PSUM") as ps_sp:

            # ---- input DMAs -------------------------------------------------
            # small/constant tensors on the DVE queue (SP queue carries the
            # big streams; keeps them out of each other's way)
            tqb = consts.tile([128, 3], f32, tag="qb")
            nc.scalar.dma_start(tqb[:], qb[:])
            tsm = consts.tile([128, 3], f16, tag="smalls")
            nc.scalar.dma_start(tsm[:], smalls[:])
            tor = consts.tile([1, 3 * 128], f16, tag="rows")
            nc.scalar.dma_start(tor[:], rows[:])
            tmp = consts.tile([1, N], f32, tag="mP")
            nc.scalar.dma_start(tmp[:], mP[:])
            tmask = consts.tile([128, NT * NSLOT, 128], f16, tag="masks")
            nc.scalar.dma_start(tmask[:], masks[:])

            # big streams, in consumption order, halves for early PE start
            twq = consts.tile([128, NCT, 2 * DH], f16, tag="wq")
            nc.sync.dma_start(twq[:], wq[:])
            tq = consts.tile([128, NCT, N], f16, tag="actQ")
            nc.sync.dma_start(tq[:, 0:4, :], actQ[:, 0:4, :])
            nc.sync.dma_start(tq[:, 4:8, :], actQ[:, 4:8, :])
            twk = consts.tile([128, NCT, 2 * DH], f16, tag="wk")
            nc.sync.dma_start(twk[:], wk[:])
            tc_ = consts.tile([128, NCT, N], f16, tag="actC")
            nc.sync.dma_start(tc_[:, 0:4, :], actC[:, 0:4, :])
            nc.sync.dma_start(tc_[:, 4:8, :], actC[:, 4:8, :])
            twv = consts.tile([128, NCT, 2 * DH], f16, tag="wv")
            nc.sync.dma_start(twv[:], wv[:])
            two = consts.tile([128, 4, DM], f16, tag="worw")
            nc.sync.dma_start(two[:], worw[:])

            # ---- projections ------------------------------------------------
            # qT / kT: [dh, n] transposed layout, per head
            qT = work.tile([128, 2, N], f16, tag="qT")
            kT = work.tile([128, 2, N], f16, tag="kT")
            for h in range(2):
                ps = ps_big.tile([128, N], f32, tag="big")
                for ct in range(NCT):
                    nc.tensor.matmul(ps[:], twq[:, ct, h * DH:(h + 1) * DH],
                                     tq[:, ct, :],
                                     start=(ct == 0), stop=(ct == NCT - 1))
                nc.scalar.activation(qT[:, h, :], ps[:], Ident,
                                     bias=tqb[:, h:h + 1])
            for h in range(2):
                ps = ps_big.tile([128, N], f32, tag="big")
                for ct in range(NCT):
                    nc.tensor.matmul(ps[:], twk[:, ct, h * DH:(h + 1) * DH],
                                     tc_[:, ct, :],
                                     start=(ct == 0), stop=(ct == NCT - 1))
                nc.vector.tensor_copy(kT[:, h, :], ps[:])

            # V natural [j, d] per head, j-tiles along free dim
            vN = work.tile([128, 2, NT * DH], f16, tag="vN")
            for h in range(2):
                ps = ps_big.tile([128, N], f32, tag="big")
                for jt in range(NT):
                    blk = ps[:, jt * DH:(jt + 1) * DH]
                    for ct in range(NCT):
                        nc.tensor.matmul(
                            blk, tc_[:, ct, jt * 128:(jt + 1) * 128],
                            twv[:, ct, h * DH:(h + 1) * DH],
                            start=(ct == 0), stop=(ct == NCT - 1))
                if h == 0:
                    nc.vector.tensor_copy(vN[:, 0, :], ps[:])
                else:
                    nc.scalar.activation(vN[:, 1, :], ps[:], Copy)

            # ---- banded attention ------------------------------------------
            onorm = work.tile([128, 2, N], f16, tag="onorm")
            # matmul outs need partition base 0/32/64: head h row at 32*h
            ssum = ps_ssum.tile([33, N], f32, tag="ssum")
            for h in range(2):
                eTm = work.tile([128, NT * NSLOT, 128], f16, tag=f"eTm{h}")
                # pad-slot score row: sp = q . (kb - kbf), then exp * npad
                psp = ps_sp.tile([1, N], f32, tag="sp")
                nc.tensor.matmul(psp[:], tsm[:, 1 + h:2 + h], qT[:, h, :],
                                 start=True, stop=True)
                eP = small.tile([1, N], f16, tag="eP")
                nc.scalar.activation(eP[:], psp[:], Exp, bias=tqb[0:1, 2:3])
                ePm = small.tile([1, N], f16, tag="ePm")
                nc.vector.tensor_mul(ePm[:], eP[:], tmp[:])

                oT = ps_att.tile([128, N], f32, tag="oT")
                sh = ssum[32 * h:32 * h + 1, :]

                # pad terms open the full-width accumulation groups:
                # denom group with npad*exp(sp), AV group with vbd x ePm
                nc.tensor.matmul(sh[:], tsm[0:1, 0:1], ePm[:],
                                 start=True, stop=False)
                nc.tensor.matmul(oT[:], tor[:, (1 + h) * 128:(2 + h) * 128],
                                 ePm[:], start=True, stop=False)

                def scores(it):
                    st = ps_st.tile([128, NSLOT, 128], f32, tag="st")
                    for s in range(NSLOT):
                        js = JSUB[it][s]
                        nc.tensor.matmul(
                            st[:, s, :],
                            kT[:, h, js * 128:(js + 1) * 128],
                            qT[:, h, it * 128:(it + 1) * 128],
                            start=True, stop=True)
                    eT = work.tile([128, NSLOT, 128], f16, tag="eT")
                    nc.scalar.activation(eT[:], st[:], Exp, bias=tqb[:, 2:3])
                    base = it * NSLOT
                    nc.vector.tensor_mul(eTm[:, base:base + NSLOT, :], eT[:],
                                         tmask[:, base:base + NSLOT, :])

                def consume(it):
                    base = it * NSLOT
                    iw = slice(it * 128, (it + 1) * 128)
                    last = it == NT - 1
                    for s in range(NSLOT):
                        nc.tensor.matmul(sh[:, iw], tsm[:, 0:1],
                                         eTm[:, base + s, :],
                                         start=False,
                                         stop=(last and s == NSLOT - 1))
                    for s in range(NSLOT):
                        js = JSUB[it][s]
                        nc.tensor.matmul(oT[:, iw],
                                         vN[:, h, js * DH:(js + 1) * DH],
                                         eTm[:, base + s, :],
                                         start=False,
                                         stop=(last and s == NSLOT - 1))

                scores(0)
                for it in range(1, NT):
                    scores(it)
                    consume(it - 1)
                consume(NT - 1)

                # broadcast f16 denom to 128 partitions via PE, then take
                # the reciprocal in f32 (1/denom can overflow f16)
                dsum = small.tile([1, N], f16, tag="dsum")
                with nc.allow_low_precision(
                        reason="denom in [4e-5, 2e4] after the -10 score "
                               "shift; f16 keeps 0.05% rel err"):
                    nc.vector.tensor_copy(dsum[:], sh)
                rbc = ps_big.tile([128, N], f32, tag="big")
                nc.tensor.matmul(rbc[:], tor[:, 0:128], dsum[:], start=True,
                                 stop=True)
                rsb = work.tile([128, N], f32, tag="rsb")
                nc.vector.reciprocal(rsb[:], rbc[:])
                nc.vector.tensor_mul(onorm[:, h, :], oT[:], rsb[:])

            # ---- o-projection + residual, fused PSUM accumulation ----------
            for dt_i in range(DM // 128):
                dw = slice(dt_i * 128, (dt_i + 1) * 128)
                pp = ps_big.tile([128, N], f32, tag="big")
                nc.tensor.matmul(pp[:], two[:, 0, dw], onorm[:, 0, :],
                                 start=True, stop=False)
                nc.tensor.matmul(pp[:], two[:, 1, dw], onorm[:, 1, :],
                                 start=False, stop=False)
                nc.tensor.matmul(pp[:], two[:, 2, dw], tq[:, 0, :],
                                 start=False, stop=False)
                nc.tensor.matmul(pp[:], two[:, 3, dw], tq[:, 1, :],
                                 start=False, stop=True)
                osb = po_pool.tile([128, N], f16, tag="po_sb")
                if dt_i % 2 == 0:
                    nc.vector.tensor_copy(osb[:], pp[:])
                else:
                    nc.scalar.activation(osb[:], pp[:], Copy)
                nc.gpsimd.dma_start(po[:, dt_i, :], osb[:])

    nc.compile()
    return nc


def _band_masks():
    """bf16 mask blocks [128, NT*NSLOT, 128] and npad row [1, N]."""
    i = np.arange(N)
    lo = np.maximum(i - WINDOW // 2, 0)
    hi = np.minimum(i + WINDOW // 2 + 1, N - 1)
    npad = np.maximum(0, WINDOW - (hi - lo + 1)).astype(np.float64)

    m = np.zeros((128, NT * NSLOT, 128), dtype=np.float64)
    for it in range(NT):
        iv = 128 * it + np.arange(128)[None, :]
        seen = set()
        for s in range(NSLOT):
            js = JSUB[it][s]
            if js in seen:
                continue
            seen.add(js)
            jv = 128 * js + np.arange(128)[:, None]
            m[:, it * NSLOT + s, :] = ((jv >= iv - WINDOW // 2)
                                       & (jv <= iv + WINDOW // 2 + 1))
    return m.astype(F16), npad[None, :]


def _pad_rows(a, rows):
    out = np.zeros((rows, a.shape[1]), dtype=a.dtype)
    out[:a.shape[0]] = a
    return out


def kernel(**inputs):
    from concourse.bass_utils import run_bass_kernel_spmd

    if "prog" not in _prog_cache:
        _prog_cache["prog"] = _build_program()
    nc = _prog_cache["prog"]

    f8 = lambda x: np.asarray(x, dtype=np.float64)
    images = f8(inputs["images"])[0]        # [N, 1024]
    caps = f8(inputs["capitions"])[0]       # [N, 768]
    ip_w, ip_b = f8(inputs["ip_w"]), f8(inputs["ip_b"])
    tp_w, tp_b = f8(inputs["tp_w"]), f8(inputs["tp_b"])

    sc = 1.0 / math.sqrt(DH)
    mM, mP = _band_masks()
    xTi = images.T                          # [1024, N]
    xTt_pad = _pad_rows(caps.T, DM)         # [1024, N], rows 768+ zero

    # per-module folded params
    mod = {}
    for m, pw, pb, cw, cb in (("ia", ip_w, ip_b, tp_w, tp_b),
                              ("ta", tp_w, tp_b, ip_w, ip_b)):
        qw, qb_ = f8(inputs[f"{m}_qw"]), f8(inputs[f"{m}_qb"])
        kw, kb_ = f8(inputs[f"{m}_kw"]), f8(inputs[f"{m}_kb"])
        vw, vb_ = f8(inputs[f"{m}_vw"]), f8(inputs[f"{m}_vb"])
        ow, ob_ = f8(inputs[f"{m}_ow"]), f8(inputs[f"{m}_ob"])
        wqf = _pad_rows(((qw @ pw) * sc).T, DM)      # [1024, 256*4] rows=chunks
        qbf = (qw @ pb + qb_) * sc                   # [1024]
        wkf = _pad_rows((kw @ cw).T, DM)
        kbf = kw @ cb + kb_
        kbd = kb_ - kbf                              # pad-score vector
        wvf = _pad_rows((vw @ cw).T, DM)
        vbf = vw @ cb + vb_
        vbd = vb_ - vbf
        # host constant: ob + ow @ vbf (v-bias fold) + resid bias pb
        cvec = ob_ + ow @ vbf + pb
        mod[m] = dict(wqf=wqf, qbf=qbf, wkf=wkf, kbd=kbd, wvf=wvf, vbd=vbd,
                      ow=ow, cvec=cvec, pw=pw)

    # residual contraction chunks per core: ia cores get 2 of ip_w's 8;
    # ta cores get 2,2,1,1 of tp_w's 6 (padded with zeros).
    resid_chunks = {0: [(0, 1), (2, 3), (4, 5), (6, 7)],
                    1: [(0, 1), (2, 3), (4, None), (5, None)]}

    def to3(a, dt=F16):
        # [1024, X] -> [128, 8, X]
        return np.ascontiguousarray(
            a.reshape(NCT, 128, -1).transpose(1, 0, 2)).astype(dt)

    in_maps = []
    for core in range(NCORES):
        mi = core // 4          # 0 -> ia, 1 -> ta
        c = core % 4
        m = mod["ia" if mi == 0 else "ta"]
        xq = xTi if mi == 0 else xTt_pad
        xc = xTt_pad if mi == 0 else xTi
        hp = slice(2 * c * DH, (2 * c + 2) * DH)     # this core's 2 heads

        # chunk permutation: resid chunks first
        chunks = resid_chunks[mi][c]
        order = [ch for ch in chunks if ch is not None]
        order += [ch for ch in range(NCT) if ch not in order]
        perm = np.concatenate([np.arange(ch * 128, (ch + 1) * 128)
                               for ch in order])

        wo2 = m["ow"][:, hp].T.reshape(2, 128, DM)   # per-head o-proj slices
        rw = np.zeros((2, 128, DM))
        for k_i, ch in enumerate(chunks):
            if ch is not None:
                rw[k_i] = m["pw"].T[ch * 128:(ch + 1) * 128, :]
        worw = np.concatenate([wo2, rw], axis=0).transpose(1, 0, 2)

        sm = np.zeros((128, 3))
        sm[:, 0] = 1.0                               # ones col (denom lhsT)
        sm[:, 1] = m["kbd"][hp][0 * DH:1 * DH]
        sm[:, 2] = m["kbd"][hp][1 * DH:2 * DH]
        rows = np.zeros((1, 3 * 128))
        rows[0, 0:128] = 1.0                         # ones row (rinv bcast)
        rows[0, 128:256] = m["vbd"][hp][0 * DH:1 * DH]
        rows[0, 256:384] = m["vbd"][hp][1 * DH:2 * DH]

        im = {
            "actQ": to3(xq[perm]),
            "actC": to3(xc),
            "wq": to3(m["wqf"][perm][:, hp]),
            "wk": to3(m["wkf"][:, hp]),
            "wv": to3(m["wvf"][:, hp]),
            "worw": np.ascontiguousarray(worw).astype(F16),
            "masks": mM,
            "mP": mP.astype(np.float32),
            "qb": np.ascontiguousarray(np.concatenate(
                [m["qbf"][hp].reshape(2, 128).T,
                 np.full((128, 1), SHIFT)], axis=1)).astype(np.float32),
            "smalls": np.ascontiguousarray(sm).astype(F16),
            "rows": np.ascontiguousarray(rows).astype(F16),
        }
        in_maps.append(im)

    res = run_bass_kernel_spmd(nc, in_maps, list(range(NCORES)))
    LAST_RESULT["res"] = res

    outs = []
    for mi, mname in ((0, "ia"), (1, "ta")):
        acc = np.zeros((DM, N), dtype=np.float64)
        for c in range(4):
            r = res.results[mi * 4 + c]
            acc += r["po"].astype(np.float64).transpose(1, 0, 2) \
                .reshape(DM, N)
        acc += mod[mname]["cvec"][:, None]
        outs.append(np.ascontiguousarray(acc.T[None]).astype(np.float32))
    return outs[0], outs[1]


# revision 13
# speedup vs baseline: 1.8609x; 1.0254x over previous
"""Trainium2 Bass kernel for nn_CrossModalFusionModel (sliding-window
cross-attention, 2 modules: image<-text and text<-image).

Sharding v2: module-split tensor parallelism over 8 NeuronCores. Cores 0-3
own the image-attention module (2 heads each), cores 4-7 the text-attention
module. Each core computes its 2 heads' folded Q/K/V projections (input-proj
chain collapsed into one matmul), banded sliding-window attention (scores /
softmax / AV restricted to 128x128 blocks on the 66-wide diagonal band), and
a full-D o-projection partial with the residual input-projection folded into
the same PSUM accumulation. The host sums the 4 partials per module.

Precision: f16 weights/activations (PE full rate, half the DMA bytes of f32);
bf16 for the exp/E path (exp(score) can exceed f16 max) and for 1/denom
(avoids f16 underflow); f32 PSUM accumulation everywhere.

Bias algebra: the folded k-bias adds a per-query constant to every real score
and the pad slots can be shifted to match (softmax-invariant), so K carries
no bias; the pad-slot score becomes q.(kb - kbf), one [1,N] matmul. The
folded v-bias contributes vbf + (vb - vbf) * (npad * exp(sp) / denom) to the
normalized per-head output; the constant vbf part goes to the host, the
rank-1 part is one accumulating matmul into the AV PSUM per head.
"""

import math

import numpy as np
import ml_dtypes

N = 512          # tokens / patches (both modalities)
DM = 1024        # d_model
DH = 128         # head dim
NT = N // 128    # 4 tiles of 128
C_IMG = 1024
C_TXT = 768
WINDOW = 64
NCORES = 8
NCT = 8          # contraction tiles (txt side zero-padded 768 -> 1024)

F16 = np.float16
BF16 = ml_dtypes.bfloat16

# j-subtile coverage per query i-tile: queries [128it, 128it+128) attend
# j in [128it-32, 128it+160), i.e. aligned j-tiles {it-1, it, it+1} clamped.
# 3 slots per it; duplicate a real jsub in dead slots (mask zeroes them).
JSUB = [(0, 1, 1), (0, 1, 2), (1, 2, 3), (2, 3, 3)]
NSLOT = 3
SHIFT = -10.0     # exp(score + SHIFT): softmax-invariant f16 range guard

_prog_cache = {}
LAST_RESULT = {}
COMPUTE_DTYPE = "f16+bf16"   # fixed; kept for the test harness banner


def _build_program():
    import concourse.tile as tile
    from concourse import bacc, mybir

    f32 = mybir.dt.float32
    f16 = mybir.dt.float16
    bf16 = mybir.dt.bfloat16
    Exp = mybir.ActivationFunctionType.Exp
    Ident = mybir.ActivationFunctionType.Identity
    Copy = mybir.ActivationFunctionType.Copy

    nc = bacc.Bacc("TRN2", target_bir_lowering=False, debug=False,
                   num_devices=NCORES)

    def din(name, shape, dt=f16):
        return nc.dram_tensor(name, shape, dt, kind="ExternalInput")

    # q-side activations, chunk-permuted so this core's 2 residual
    # contraction chunks sit at chunk indices 0 and 1.
    actQ = din("actQ", [128, NCT, N])
    actC = din("actC", [128, NCT, N])
    wq = din("wq", [128, NCT, 2 * DH])      # folded + scaled, rows match actQ
    wk = din("wk", [128, NCT, 2 * DH])
    wv = din("wv", [128, NCT, 2 * DH])
    worw = din("worw", [128, 4, DM])        # wo head0, wo head1, rw ch0, rw ch1
    masks = din("masks", [128, NT * NSLOT, 128], f16)
    mP = din("mP", [1, N], f32)             # npad per query
    qb = din("qb", [128, 3], f32)   # col 2: exp shift const           # per-head folded q bias
    smalls = din("smalls", [128, 3], f16)  # onesc | kbd h0 | kbd h1
    rows = din("rows", [1, 3 * 128], f16)  # ones row | vbd h0 | vbd h1
    po = nc.dram_tensor("po", [128, DM // 128, N], f16, kind="ExternalOutput")

    with tile.TileContext(nc) as tc:
        with tc.tile_pool(name="consts", bufs=1) as consts, \
             tc.tile_pool(name="work", bufs=3) as work, \
             tc.tile_pool(name="po_sb", bufs=3) as po_pool, \
             tc.tile_pool(name="small", bufs=2) as small, \
             tc.tile_pool(name="ps_big", bufs=2, space="PSUM") as ps_big, \
             tc.tile_pool(name="ps_st", bufs=2, space="PSUM") as ps_st, \
             tc.tile_pool(name="ps_att", bufs=2, space="PSUM") as ps_att, \
             tc.tile_pool(name="ps_ssum", bufs=1, space="PSUM") as ps_ssum, \
             tc.tile_pool(name="ps_sp", bufs=1, space="PSUM") as ps_sp:

            # ---- input DMAs -------------------------------------------------
            # small/constant tensors on the DVE queue (SP queue carries the
            # big streams; keeps them out of each other's way)
            tqb = consts.tile([128, 3], f32, tag="qb")
            nc.scalar.dma_start(tqb[:], qb[:])
            tsm = consts.tile([128, 3], f16, tag="smalls")
            nc.scalar.dma_start(tsm[:], smalls[:])
            tor = consts.tile([1, 3 * 128], f16, tag="rows")
            nc.scalar.dma_start(tor[:], rows[:])
            tmp = consts.tile([1, N], f32, tag="mP")
            nc.scalar.dma_start(tmp[:], mP[:])
            tmask = consts.tile([128, NT * NSLOT, 128], f16, tag="masks")
            nc.scalar.dma_start(tmask[:], masks[:])

            # big streams, in consumption order, halves for early PE start
            twq = consts.tile([128, NCT, 2 * DH], f16, tag="wq")
            tq = consts.tile([128, NCT, N], f16, tag="actQ")
            nc.sync.dma_start(twq[:, 0:4, :], wq[:, 0:4, :])
            nc.sync.dma_start(tq[:, 0:2, :], actQ[:, 0:2, :])
            nc.sync.dma_start(tq[:, 2:4, :], actQ[:, 2:4, :])
            nc.sync.dma_start(twq[:, 4:8, :], wq[:, 4:8, :])
            nc.sync.dma_start(tq[:, 4:6, :], actQ[:, 4:6, :])
            nc.sync.dma_start(tq[:, 6:8, :], actQ[:, 6:8, :])
            twk = consts.tile([128, NCT, 2 * DH], f16, tag="wk")
            nc.sync.dma_start(twk[:], wk[:])
            tc_ = consts.tile([128, NCT, N], f16, tag="actC")
            nc.sync.dma_start(tc_[:, 0:2, :], actC[:, 0:2, :])
            nc.sync.dma_start(tc_[:, 2:4, :], actC[:, 2:4, :])
            twv = consts.tile([128, NCT, 2 * DH], f16, tag="wv")
            nc.sync.dma_start(tc_[:, 4:6, :], actC[:, 4:6, :])
            nc.sync.dma_start(tc_[:, 6:8, :], actC[:, 6:8, :])
            nc.sync.dma_start(twv[:], wv[:])
            two = consts.tile([128, 4, DM], f16, tag="worw")
            nc.sync.dma_start(two[:], worw[:])

            # ---- projections ------------------------------------------------
            # qT / kT: [dh, n] transposed layout, per head; ct-major loops so
            # the first matmuls only need the first DMA chunks
            qT = work.tile([128, 2, N], f16, tag="qT")
            kT = work.tile([128, 2, N], f16, tag="kT")
            qps = [ps_big.tile([128, N], f32, tag="big", name=f"qps{h}")
                   for h in range(2)]
            for ct in range(NCT):
                for h in range(2):
                    nc.tensor.matmul(qps[h][:],
                                     twq[:, ct, h * DH:(h + 1) * DH],
                                     tq[:, ct, :],
                                     start=(ct == 0), stop=(ct == NCT - 1))
            for h in range(2):
                nc.scalar.activation(qT[:, h, :], qps[h][:], Ident,
                                     bias=tqb[:, h:h + 1])
            kps = [ps_big.tile([128, N], f32, tag="big", name=f"kps{h}")
                   for h in range(2)]
            for ct in range(NCT):
                for h in range(2):
                    nc.tensor.matmul(kps[h][:],
                                     twk[:, ct, h * DH:(h + 1) * DH],
                                     tc_[:, ct, :],
                                     start=(ct == 0), stop=(ct == NCT - 1))
            for h in range(2):
                nc.vector.tensor_copy(kT[:, h, :], kps[h][:])

            # pad-slot score rows early (hides the exp+mul latency behind
            # the V projection): sp = q . (kb - kbf), exp(+shift) * npad
            ePms = []
            for h in range(2):
                psp = ps_sp.tile([1, N], f32, tag="sp")
                nc.tensor.matmul(psp[:], tsm[:, 1 + h:2 + h], qT[:, h, :],
                                 start=True, stop=True)
                eP = small.tile([1, N], f16, tag="eP")
                nc.scalar.activation(eP[:], psp[:], Exp, bias=tqb[0:1, 2:3])
                ePm = small.tile([1, N], f16, tag="ePm")
                nc.vector.tensor_mul(ePm[:], eP[:], tmp[:])
                ePms.append(ePm)

            # V natural [j, d] per head, j-tiles along free dim
            vN = work.tile([128, 2, NT * DH], f16, tag="vN")
            for h in range(2):
                ps = ps_big.tile([128, N], f32, tag="big")
                for jt in range(NT):
                    blk = ps[:, jt * DH:(jt + 1) * DH]
                    for ct in range(NCT):
                        nc.tensor.matmul(
                            blk, tc_[:, ct, jt * 128:(jt + 1) * 128],
                            twv[:, ct, h * DH:(h + 1) * DH],
                            start=(ct == 0), stop=(ct == NCT - 1))
                if h == 0:
                    nc.vector.tensor_copy(vN[:, 0, :], ps[:])
                else:
                    nc.scalar.activation(vN[:, 1, :], ps[:], Copy)

            # ---- banded attention (heads interleaved) ----------------------
            onorm = work.tile([128, 2, N], f16, tag="onorm")
            # matmul outs need partition base 0/32/64: head h row at 32*h
            ssum = ps_ssum.tile([33, N], f32, tag="ssum")
            eTms = [work.tile([128, NT * NSLOT, 128], f16, tag=f"eTm{h}",
                             name=f"eTm{h}") for h in range(2)]
            oTs = [None, None]

            def openers(h):
                # pad terms open the full-width accumulation groups:
                # denom group with npad*exp(sp), AV group with vbd x ePm
                oTs[h] = ps_att.tile([128, N], f32, tag="oT", name=f"oT{h}")
                sh = ssum[32 * h:32 * h + 1, :]
                nc.tensor.matmul(sh[:], tsm[0:1, 0:1], ePms[h][:],
                                 start=True, stop=False)
                nc.tensor.matmul(oTs[h][:],
                                 tor[:, (1 + h) * 128:(2 + h) * 128],
                                 ePms[h][:], start=True, stop=False)

            def scores(h, it):
                st = ps_st.tile([128, NSLOT, 128], f32, tag="st")
                for s in range(NSLOT):
                    js = JSUB[it][s]
                    nc.tensor.matmul(
                        st[:, s, :],
                        kT[:, h, js * 128:(js + 1) * 128],
                        qT[:, h, it * 128:(it + 1) * 128],
                        start=True, stop=True)
                eT = work.tile([128, NSLOT, 128], f16, tag="eT")
                nc.scalar.activation(eT[:], st[:], Exp, bias=tqb[:, 2:3])
                base = it * NSLOT
                nc.vector.tensor_mul(eTms[h][:, base:base + NSLOT, :], eT[:],
                                     tmask[:, base:base + NSLOT, :])

            def consume(h, it):
                base = it * NSLOT
                iw = slice(it * 128, (it + 1) * 128)
                last = it == NT - 1
                sh = ssum[32 * h:32 * h + 1, :]
                for s in range(NSLOT):
                    nc.tensor.matmul(sh[:, iw], tsm[:, 0:1],
                                     eTms[h][:, base + s, :],
                                     start=False,
                                     stop=(last and s == NSLOT - 1))
                for s in range(NSLOT):
                    js = JSUB[it][s]
                    nc.tensor.matmul(oTs[h][:, iw],
                                     vN[:, h, js * DH:(js + 1) * DH],
                                     eTms[h][:, base + s, :],
                                     start=False,
                                     stop=(last and s == NSLOT - 1))

            def finish(h):
                # broadcast f16 denom to 128 partitions via PE, then take
                # the reciprocal in f32 (1/denom can overflow f16)
                dsum = small.tile([1, N], f16, tag="dsum")
                with nc.allow_low_precision(
                        reason="denom in [4e-5, 2e4] after the -10 score "
                               "shift; f16 keeps 0.05% rel err"):
                    nc.vector.tensor_copy(dsum[:], ssum[32 * h:32 * h + 1, :])
                rbc = ps_big.tile([128, N], f32, tag="big")
                nc.tensor.matmul(rbc[:], tor[:, 0:128], dsum[:], start=True,
                                 stop=True)
                rsb = work.tile([128, N], f32, tag="rsb")
                nc.vector.reciprocal(rsb[:], rbc[:])
                nc.vector.tensor_mul(onorm[:, h, :], oTs[h][:], rsb[:])

            openers(0)
            scores(0, 0)
            scores(0, 1)
            consume(0, 0)
            scores(0, 2)
            consume(0, 1)
            scores(0, 3)
            consume(0, 2)
            consume(0, 3)
            openers(1)
            scores(1, 0)
            scores(1, 1)
            finish(0)
            consume(1, 0)
            scores(1, 2)
            consume(1, 1)
            scores(1, 3)
            consume(1, 2)
            consume(1, 3)
            finish(1)

            # ---- o-projection + residual, fused PSUM accumulation ----------
            for dt_i in range(DM // 128):
                dw = slice(dt_i * 128, (dt_i + 1) * 128)
                pp = ps_big.tile([128, N], f32, tag="big")
                nc.tensor.matmul(pp[:], two[:, 0, dw], onorm[:, 0, :],
                                 start=True, stop=False)
                nc.tensor.matmul(pp[:], two[:, 1, dw], onorm[:, 1, :],
                                 start=False, stop=False)
                nc.tensor.matmul(pp[:], two[:, 2, dw], tq[:, 0, :],
                                 start=False, stop=False)
                nc.tensor.matmul(pp[:], two[:, 3, dw], tq[:, 1, :],
                                 start=False, stop=True)
                osb = po_pool.tile([128, N], f16, tag="po_sb")
                if dt_i % 2 == 0:
                    nc.vector.tensor_copy(osb[:], pp[:])
                else:
                    nc.scalar.activation(osb[:], pp[:], Copy)
                nc.sync.dma_start(po[:, dt_i, :], osb[:])

    nc.compile()
    return nc


def _band_masks():
    """bf16 mask blocks [128, NT*NSLOT, 128] and npad row [1, N]."""
    i = np.arange(N)
    lo = np.maximum(i - WINDOW // 2, 0)
    hi = np.minimum(i + WINDOW // 2 + 1, N - 1)
    npad = np.maximum(0, WINDOW - (hi - lo + 1)).astype(np.float64)

    m = np.zeros((128, NT * NSLOT, 128), dtype=np.float64)
    for it in range(NT):
        iv = 128 * it + np.arange(128)[None, :]
        seen = set()
        for s in range(NSLOT):
            js = JSUB[it][s]
            if js in seen:
                continue
            seen.add(js)
            jv = 128 * js + np.arange(128)[:, None]
            m[:, it * NSLOT + s, :] = ((jv >= iv - WINDOW // 2)
                                       & (jv <= iv + WINDOW // 2 + 1))
    return m.astype(F16), npad[None, :]


def _pad_rows(a, rows):
    out = np.zeros((rows, a.shape[1]), dtype=a.dtype)
    out[:a.shape[0]] = a
    return out


def kernel(**inputs):
    from concourse.bass_utils import run_bass_kernel_spmd

    if "prog" not in _prog_cache:
        _prog_cache["prog"] = _build_program()
    nc = _prog_cache["prog"]

    f8 = lambda x: np.asarray(x, dtype=np.float64)
    images = f8(inputs["images"])[0]        # [N, 1024]
    caps = f8(inputs["capitions"])[0]       # [N, 768]
    ip_w, ip_b = f8(inputs["ip_w"]), f8(inputs["ip_b"])
    tp_w, tp_b = f8(inputs["tp_w"]), f8(inputs["tp_b"])

    sc = 1.0 / math.sqrt(DH)
    mM, mP = _band_masks()
    xTi = images.T                          # [1024, N]
    xTt_pad = _pad_rows(caps.T, DM)         # [1024, N], rows 768+ zero

    # per-module folded params
    mod = {}
    for m, pw, pb, cw, cb in (("ia", ip_w, ip_b, tp_w, tp_b),
                              ("ta", tp_w, tp_b, ip_w, ip_b)):
        qw, qb_ = f8(inputs[f"{m}_qw"]), f8(inputs[f"{m}_qb"])
        kw, kb_ = f8(inputs[f"{m}_kw"]), f8(inputs[f"{m}_kb"])
        vw, vb_ = f8(inputs[f"{m}_vw"]), f8(inputs[f"{m}_vb"])
        ow, ob_ = f8(inputs[f"{m}_ow"]), f8(inputs[f"{m}_ob"])
        wqf = _pad_rows(((qw @ pw) * sc).T, DM)      # [1024, 256*4] rows=chunks
        qbf = (qw @ pb + qb_) * sc                   # [1024]
        wkf = _pad_rows((kw @ cw).T, DM)
        kbf = kw @ cb + kb_
        kbd = kb_ - kbf                              # pad-score vector
        wvf = _pad_rows((vw @ cw).T, DM)
        vbf = vw @ cb + vb_
        vbd = vb_ - vbf
        # host constant: ob + ow @ vbf (v-bias fold) + resid bias pb
        cvec = ob_ + ow @ vbf + pb
        mod[m] = dict(wqf=wqf, qbf=qbf, wkf=wkf, kbd=kbd, wvf=wvf, vbd=vbd,
                      ow=ow, cvec=cvec, pw=pw)

    # residual contraction chunks per core: ia cores get 2 of ip_w's 8;
    # ta cores get 2,2,1,1 of tp_w's 6 (padded with zeros).
    resid_chunks = {0: [(0, 1), (2, 3), (4, 5), (6, 7)],
                    1: [(0, 1), (2, 3), (4, None), (5, None)]}

    def to3(a, dt=F16):
        # [1024, X] -> [128, 8, X]
        return np.ascontiguousarray(
            a.reshape(NCT, 128, -1).transpose(1, 0, 2)).astype(dt)

    in_maps = []
    for core in range(NCORES):
        mi = core // 4          # 0 -> ia, 1 -> ta
        c = core % 4
        m = mod["ia" if mi == 0 else "ta"]
        xq = xTi if mi == 0 else xTt_pad
        xc = xTt_pad if mi == 0 else xTi
        hp = slice(2 * c * DH, (2 * c + 2) * DH)     # this core's 2 heads

        # chunk permutation: resid chunks first
        chunks = resid_chunks[mi][c]
        order = [ch for ch in chunks if ch is not None]
        order += [ch for ch in range(NCT) if ch not in order]
        perm = np.concatenate([np.arange(ch * 128, (ch + 1) * 128)
                               for ch in order])

        wo2 = m["ow"][:, hp].T.reshape(2, 128, DM)   # per-head o-proj slices
        rw = np.zeros((2, 128, DM))
        for k_i, ch in enumerate(chunks):
            if ch is not None:
                rw[k_i] = m["pw"].T[ch * 128:(ch + 1) * 128, :]
        worw = np.concatenate([wo2, rw], axis=0).transpose(1, 0, 2)

        sm = np.zeros((128, 3))
        sm[:, 0] = 1.0                               # ones col (denom lhsT)
        sm[:, 1] = m["kbd"][hp][0 * DH:1 * DH]
        sm[:, 2] = m["kbd"][hp][1 * DH:2 * DH]
        rows = np.zeros((1, 3 * 128))
        rows[0, 0:128] = 1.0                         # ones row (rinv bcast)
        rows[0, 128:256] = m["vbd"][hp][0 * DH:1 * DH]
        rows[0, 256:384] = m["vbd"][hp][1 * DH:2 * DH]

        im = {
            "actQ": to3(xq[perm]),
            "actC": to3(xc),
            "wq": to3(m["wqf"][perm][:, hp]),
            "wk": to3(m["wkf"][:, hp]),
            "wv": to3(m["wvf"][:, hp]),
            "worw": np.ascontiguousarray(worw).astype(F16),
            "masks": mM,
            "mP": mP.astype(np.float32),
            "qb": np.ascontiguousarray(np.concatenate(
                [m["qbf"][hp].reshape(2, 128).T,
                 np.full((128, 1), SHIFT)], axis=1)).astype(np.float32),
            "smalls": np.ascontiguousarray(sm).astype(F16),
            "rows": np.ascontiguousarray(rows).astype(F16),
        }
        in_maps.append(im)

    res = run_bass_kernel_spmd(nc, in_maps, list(range(NCORES)))
    LAST_RESULT["res"] = res

    outs = []
    for mi, mname in ((0, "ia"), (1, "ta")):
        acc = np.zeros((DM, N), dtype=np.float64)
        for c in range(4):
            r = res.results[mi * 4 + c]
            acc += r["po"].astype(np.float64).transpose(1, 0, 2) \
                .reshape(DM, N)
        acc += mod[mname]["cvec"][:, None]
        outs.append(np.ascontiguousarray(acc.T[None]).astype(np.float32))
    return outs[0], outs[1]
